# revision 21
# baseline (speedup 1.0000x reference)
"""Bahdanau additive-attention kernel for Trainium2, SPMD across 8 NeuronCores.

Reference computation (all fp32):
    q_proj  = query @ W1_w.T + W1_b            # [D]
    v_proj  = values @ W2_w.T + W2_b           # [T, D]
    weights = softmax(tanh(q_proj + v_proj) * v, axis=0)   # over T
    out     = weights * values                 # [T, D]

Sharding: values is split along T across 8 cores (2048 rows each); W2 is
replicated.  The q-projection (a [D]x[D,D] matvec, 0.006% of the FLOPs) is
folded into the tanh bias on the host together with W1_b + W2_b.  Logits
are bounded in [-0.1, 0.1] (tanh * v with |v| <= 0.1) so the softmax needs
no max-subtraction pass.

Device program per core:
  - v_proj matmul in fp8 DoubleRow perf mode (2 k-tiles per instruction,
    fp8 peak rate): stationary = 32*W2 blocked fp8, moving = values^T fp8,
    fp32 PSUM.  The 1/32 de-scale rides the tanh activation's `scale`.
  - ScalarE: tanh(psum/32 + qb[d]) -> st fp16, then exp(v[d]*st) -> e fp16
    with accum_out giving the per-partition sum of exps.
  - DVE (during pass 1, needs no softmax denom): e *= valuesT fp16.
  - Sum-exp AllReduce in 2 parts; part A overlaps the tail matmuls, part B
    is a minimal-latency tail collective.
  - Tail: e *= 2^14/S[d] (DVE tensor_scalar / ScalarE copy-scale split),
    DMA out as fp16 in [d, t] layout.
Host: transpose [d,t] -> [t,d], de-scale 2^-14, cast fp32, concat shards.

DMA choreography: vt8 (matmul operand) owns both hwdge queues at t=0; the
8 MB fp16 copy of valuesT is held back by tiny dependency markers so it
doesn't steal HBM bandwidth from the critical path.
"""

import os

import numpy as np

import concourse.bacc as bacc
import concourse.bass as bass
import concourse.tile as tile
from concourse import mybir
from concourse.bass_utils import run_bass_kernel_spmd

F32 = mybir.dt.float32
FP16 = mybir.dt.float16
FP8 = mybir.dt.float8e4

D = 2048          # feature dim
T = 16384         # total timesteps
N_CORES = 8
TS = T // N_CORES  # timesteps per core = 2048

DT = D // 128      # 16 d-tiles of 128
KT = D // 128      # 16 k-tiles of 128
KSP = KT // 2      # 8 DoubleRow k-steps (2 k-tiles each)
TC = TS // 512     # 4 t-chunks of 512

# DoubleRowSwInterleave: weights pre-interleaved/reversed on host so the
# PE reads them contiguously (faster LDWEIGHTS than plain DoubleRow)
USE_SWI = os.environ.get("K_SWI", "0") == "1"

W2_SCALE = 32.0            # pre-scale on W2 so fp8 stays in normal range
INV_W2_SCALE = 1.0 / 32.0  # folded into the tanh activation's scale
OUT_SCALE = 2.0 ** 14      # keeps e/S in healthy fp16 range on device
INV_OUT_SCALE = 2.0 ** -14


def build_kernel(debug=False):
    nc = bacc.Bacc(None, target_bir_lowering=False, debug=debug,
                   num_devices=N_CORES)

    vt8 = nc.dram_tensor("vt8", [128, KT, TS], FP8, kind="ExternalInput")
    vt16 = nc.dram_tensor("vt16", [128, DT, TS], FP16, kind="ExternalInput")
    if USE_SWI:
        w2dr = nc.dram_tensor("w2dr", [DT, 128, KSP, 256], FP8,
                              kind="ExternalInput")
    else:
        w2dr = nc.dram_tensor("w2dr", [DT, 128, KT, 128], FP8,
                              kind="ExternalInput")
    qbv_h = nc.dram_tensor("qbv_h", [128, DT], F32, kind="ExternalInput")
    vv_h = nc.dram_tensor("vv_h", [128, DT], F32, kind="ExternalInput")
    outd = nc.dram_tensor("outd", [DT, 128, TS], FP16, kind="ExternalOutput")

    # sum-exp AllReduce split: [lo, hi) d-tile ranges.  Each collective is
    # latency-bound (~8-20us); they serialize on the CC core, so parts are
    # spaced so each finishes before the next triggers, and the early
    # parts' output DMAs drain during the matmul phase, keeping HBM quiet
    # for the tail collective.
    AR_PARTS = [(0, 10), (10, 14), (14, 16)]

    with tile.TileContext(nc) as tc:
        with (
            tc.tile_pool(name="const", bufs=1) as const_pool,
            tc.tile_pool(name="vt8p", bufs=1) as vt8_pool,
            tc.tile_pool(name="vt16p", bufs=1) as vt16_pool,
            tc.tile_pool(name="e", bufs=1) as e_pool,
            tc.tile_pool(name="w2", bufs=4) as w2_pool,
            tc.tile_pool(name="st", bufs=2) as st_pool,
            tc.tile_pool(name="osb", bufs=2) as osb_pool,
            tc.tile_pool(name="psum", bufs=2, space="PSUM") as psum_pool,
            tc.tile_pool(name="dram", bufs=1, space="DRAM") as dram_pool,
        ):
            # ---------------- constants / small vectors ----------------
            qbv = const_pool.tile([128, DT], F32)
            vv = const_pool.tile([128, DT], F32)
            rv2 = const_pool.tile([128, DT], F32)   # 2^14 / S[d]
            Sloc = const_pool.tile([128, DT], F32)  # local sum-exp

            # ---------------- warmup collective (absorbs ncfw first-use) ----
            wu_in = dram_pool.tile([1, 32], F32, name="wu_in")
            wu_out = dram_pool.tile([1, 32], F32, name="wu_out")
            wuz = const_pool.tile([1, 32], F32)
            nc.vector.memset(wuz[:, :], 0.0)
            nc.gpsimd.dma_start(wu_in[:, :], wuz[:, :])
            nc.gpsimd.collective_compute(
                "AllReduce", mybir.AluOpType.add,
                replica_groups=[list(range(N_CORES))],
                ins=[wu_in.opt()], outs=[wu_out.opt()],
            )

            nc.gpsimd.dma_start(qbv[:, :], qbv_h[:, :])
            nc.gpsimd.dma_start(vv[:, :], vv_h[:, :])

            # Startup-critical inputs (w2 dj0-3 + all of vt8, ~5 MB) spread
            # over all three DMA queues, ordered by earliest need per
            # queue; remaining W2 blocks stream behind them.
            w2_shape = [128, KSP, 256] if USE_SWI else [128, KT, 128]
            w2_tiles = {}
            for dj in range(DT):
                w2_tiles[dj] = w2_pool.tile(w2_shape, FP8, tag="w2",
                                            name=f"w2b{dj}")
            vt8_sb = vt8_pool.tile([128, KT, TS], FP8, name="vt8_sb")

            def w2_load(dj, eng):
                eng.dma_start(w2_tiles[dj][:, :, :], w2dr[dj, :, :, :])

            def vt8_load(c, eng):
                eng.dma_start(vt8_sb[:, 2 * c:2 * c + 2, :],
                              vt8[:, 2 * c:2 * c + 2, :])

            # per-queue order follows earliest need by dj0's ksp schedule
            w2_load(0, nc.sync)
            w2_load(1, nc.scalar)
            vt8_load(2, nc.gpsimd)
            vt8_load(0, nc.sync)
            vt8_load(1, nc.scalar)
            w2_load(2, nc.gpsimd)
            vt8_load(3, nc.sync)
            vt8_load(4, nc.scalar)
            w2_load(3, nc.gpsimd)
            vt8_load(6, nc.sync)
            vt8_load(7, nc.scalar)
            vt8_load(5, nc.gpsimd)
            for dj in range(4, DT):
                eng = (nc.sync, nc.scalar, nc.gpsimd)[dj % 3]
                w2_load(dj, eng)

            # vt16 [p, dj, t] on gpsimd, gated per-chunk by dependency
            # markers written after tanh(dj=2c+1) so it trails the dj loop
            vt16_sb = vt16_pool.tile([128, DT, TS], FP16, name="vt16_sb")

            def vt16_load(c):
                nc.gpsimd.dma_start(vt16_sb[:, 4 * c:4 * c + 4, :],
                                    vt16[:, 4 * c:4 * c + 4, :])

            e_tiles = [e_pool.tile([128, TS], FP16, name=f"e{dj}")
                       for dj in range(DT)]

            # ---------------- AllReduce plumbing ------------------------
            s_bounce = []
            for pi, (lo, hi) in enumerate(AR_PARTS):
                sin = dram_pool.tile([128, hi - lo], F32, name=f"s_in{pi}")
                sout = dram_pool.tile([128, hi - lo], F32, name=f"s_out{pi}")
                s_bounce.append((sin, sout))

            def ar_trigger(pi):
                lo, hi = AR_PARTS[pi]
                sin, sout = s_bounce[pi]
                # bounce DMA from the scalar queue: it runs right after the
                # accumulator read on the same engine, skipping a
                # cross-engine semaphore hop on the trigger path
                nc.scalar.dma_start(sin[:, :], Sloc[:, lo:hi])
                nc.gpsimd.collective_compute(
                    "AllReduce", mybir.AluOpType.add,
                    replica_groups=[list(range(N_CORES))],
                    ins=[sin.opt()], outs=[sout.opt()],
                )

            def ar_readback(pi):
                lo, hi = AR_PARTS[pi]
                sin, sout = s_bounce[pi]
                nc.gpsimd.dma_start(rv2[:, lo:hi], sout[:, :])
                nc.vector.tensor_scalar_mul(rv2[:, lo:hi], rv2[:, lo:hi],
                                            INV_OUT_SCALE)
                nc.vector.reciprocal(rv2[:, lo:hi], rv2[:, lo:hi])

            ndma = [0]

            def pass2_scale(dj, on_scalar=False):
                # e (already e*valuesT) *= 2^14/S[d], then [d, t] fp16 out
                if on_scalar:
                    osb = osb_pool.tile([128, TS], FP16, tag="osb", name="osb")
                    nc.scalar.activation(
                        osb[:, :], e_tiles[dj][:, :],
                        mybir.ActivationFunctionType.Copy,
                        bias=0.0, scale=rv2[:, dj:dj + 1])
                    src = osb
                else:
                    nc.vector.tensor_scalar(
                        out=e_tiles[dj][:, :], in0=e_tiles[dj][:, :],
                        scalar1=rv2[:, dj:dj + 1], scalar2=None,
                        op0=mybir.AluOpType.mult)
                    src = e_tiles[dj]
                ndma[0] += 1
                eng = nc.sync if ndma[0] % 2 == 0 else nc.scalar
                eng.dma_start(outd[dj, :, :], src[:, :])

            # ---------------- pass 1: matmul + tanh + exp + e*values ----
            y_cursor = [0]      # next dj whose e *= valuesT is pending
            vt16_avail = 0      # vt16 chunks emitted so far (in d-tiles)
            for dj in range(DT):
                ps = psum_pool.tile([128, TS], F32, tag="ps", name=f"ps{dj % 2}")
                for ksp in range(KSP):
                    if USE_SWI:
                        lhsT = w2_tiles[dj][:, ksp, :].rearrange(
                            "p (m two) -> p two m", two=2)
                        pm = mybir.MatmulPerfMode.DoubleRowSwInterleave
                    else:
                        lhsT = w2_tiles[dj][:, 2 * ksp:2 * ksp + 2, :]
                        pm = mybir.MatmulPerfMode.DoubleRow
                    for tci in range(TC):
                        nc.tensor.matmul(
                            ps[:, tci * 512:(tci + 1) * 512],
                            lhsT,
                            vt8_sb[:, 2 * ksp:2 * ksp + 2,
                                   tci * 512:(tci + 1) * 512],
                            start=(ksp == 0), stop=(ksp == KSP - 1),
                            perf_mode=pm,
                        )
                st = st_pool.tile([128, TS], FP16, tag="st", name="st")
                nc.scalar.activation(
                    st[:, :], ps[:, :],
                    mybir.ActivationFunctionType.Tanh,
                    bias=qbv[:, dj:dj + 1], scale=INV_W2_SCALE,
                )
                nc.scalar.activation(
                    e_tiles[dj][:, :], st[:, :],
                    mybir.ActivationFunctionType.Exp,
                    bias=0.0, scale=vv[:, dj:dj + 1],
                    accum_out=Sloc[:, dj:dj + 1],
                )
                if dj in (2, 5, 8, 11):
                    # release the next vt16 chunk: the marker write gates
                    # the DMA (WAW) behind the dj loop so it doesn't steal
                    # startup HBM bandwidth; each load is emitted before
                    # any e*values read of its range (RAW).
                    c = (dj - 2) // 3
                    nc.scalar.copy(vt16_sb[:, 4 * c:4 * c + 1, 0:1],
                                   qbv[:, 0:1])
                    vt16_load(c)
                    vt16_avail = 4 * c + 4
                # e *= valuesT (no denom needed) while TensorE grinds on
                while y_cursor[0] <= dj and y_cursor[0] < vt16_avail:
                    j = y_cursor[0]
                    nc.vector.tensor_mul(e_tiles[j][:, :], e_tiles[j][:, :],
                                         vt16_sb[:, j, :])
                    y_cursor[0] += 1

                if dj == 9:
                    ar_trigger(0)
                elif dj == 10:
                    ar_readback(0)
                elif dj == 11:
                    for j in range(0, 5):
                        pass2_scale(j)
                elif dj == 12:
                    for j in range(5, 10):
                        pass2_scale(j)
                elif dj == 13:
                    ar_trigger(1)
                elif dj == 14:
                    ar_readback(1)
                    for j in range(10, 14):
                        pass2_scale(j)
                elif dj == DT - 1:                # dj 15
                    ar_trigger(2)

            ar_readback(2)
            pass2_scale(14, on_scalar=False)
            pass2_scale(15, on_scalar=False)

    nc.compile()
    return nc


_NC_CACHE = None


def _get_nc():
    global _NC_CACHE
    if _NC_CACHE is None:
        _NC_CACHE = build_kernel()
    return _NC_CACHE


def make_in_maps(query, values, v, W1_w, W1_b, W2_w, W2_b):
    import ml_dtypes
    qb = (query @ W1_w.T + W1_b + W2_b).astype(np.float32)
    qbv_np = np.ascontiguousarray(qb.reshape(DT, 128).T)
    vv_np = np.ascontiguousarray(v.reshape(DT, 128).T)
    # [dj, p, ks, m] = 32*W2[dj*128+m, ks*128+p]
    w2dr_np = np.ascontiguousarray(
        (W2_w.T * W2_SCALE).reshape(KT, 128, DT, 128).transpose(2, 1, 0, 3)
        .astype(ml_dtypes.float8_e4m3))
    if USE_SWI:
        # per (dj, p, ksp): [A127, B127, ..., A0, B0] where A/B are the m
        # columns of k-tiles 2*ksp / 2*ksp+1, m descending
        a = w2dr_np.reshape(DT, 128, KSP, 2, 128)[..., ::-1]
        w2dr_np = np.ascontiguousarray(
            a.transpose(0, 1, 2, 4, 3).reshape(DT, 128, KSP, 256))
    in_maps = []
    for c in range(N_CORES):
        valsT = values[c * TS:(c + 1) * TS].T          # [D, TS]
        base = np.ascontiguousarray(
            valsT.reshape(DT, 128, TS).transpose(1, 0, 2))  # [p, j, t]
        in_maps.append({
            "vt8": base.astype(ml_dtypes.float8_e4m3),
            "vt16": base.astype(np.float16),
            "w2dr": w2dr_np,
            "qbv_h": qbv_np,
            "vv_h": vv_np,
        })
    return in_maps


def kernel(query, values, v, W1_w, W1_b, W2_w, W2_b, _trace=False,
           _trace_kwargs=None):
    query = np.asarray(query, np.float32)
    values = np.asarray(values, np.float32)
    v = np.asarray(v, np.float32)
    W1_w = np.asarray(W1_w, np.float32)
    W1_b = np.asarray(W1_b, np.float32)
    W2_w = np.asarray(W2_w, np.float32)
    W2_b = np.asarray(W2_b, np.float32)

    nc = _get_nc()
    in_maps = make_in_maps(query, values, v, W1_w, W1_b, W2_w, W2_b)
    res = run_bass_kernel_spmd(
        nc, in_maps, core_ids=list(range(N_CORES)),
        trace=_trace, **(_trace_kwargs or {}),
    )
    shards = []
    for om in res.results:
        o = np.asarray(om["outd"])                      # [DT, 128, TS] fp16
        o = np.transpose(o, (2, 0, 1)).reshape(TS, D)   # [t, d]
        shards.append(o.astype(np.float32) * INV_OUT_SCALE)
    out = np.concatenate(shards, axis=0)
    if _trace:
        return out, res
    return out


if __name__ == "__main__":
    nc = build_kernel()
    print("compiled OK")


# revision 27
# speedup vs baseline: 2.9554x; 2.9554x over previous
"""Bahdanau additive-attention kernel for Trainium2, SPMD across 8 NeuronCores.

Reference computation (all fp32):
    q_proj  = query @ W1_w.T + W1_b            # [D]
    v_proj  = values @ W2_w.T + W2_b           # [T, D]
    weights = softmax(tanh(q_proj + v_proj) * v, axis=0)   # over T
    out     = weights * values                 # [T, D]

Sharding: values is split along T across 8 cores (2048 rows each); W2 is
replicated.  The q-projection (a [D]x[D,D] matvec, 0.006% of the FLOPs) is
folded into the tanh bias on the host together with W1_b + W2_b.  Logits
are bounded in [-0.1, 0.1] (tanh * v with |v| <= 0.1) so the softmax needs
no max-subtraction pass.

Device program per core:
  - v_proj matmul in fp8 DoubleRow perf mode (2 k-tiles per instruction,
    fp8 peak rate): stationary = 32*W2 blocked fp8, moving = values^T fp8,
    fp32 PSUM.  The 1/32 de-scale rides the tanh activation's `scale`.
  - ScalarE: tanh(psum/32 + qb[d]) -> st fp16, then exp(v[d]*st) -> e fp16
    with accum_out giving the per-partition sum of exps.
  - DVE (during pass 1, needs no softmax denom): e *= valuesT fp16.
  - Sum-exp AllReduce in 3 latency-bound parts spaced so the early parts'
    scale + output DMAs drain during the matmul phase and only the last
    (smallest) collective sits on the tail.
  - e *= 2^14/S[d] (DVE tensor_scalar), DMA out as fp16 in [d, t] layout.
Host: transpose [d,t] -> [t,d], de-scale 2^-14, cast fp32, concat shards.

DMA choreography: vt8 (matmul operand) owns both hwdge queues at t=0; the
8 MB fp16 copy of valuesT is held back by tiny dependency markers so it
doesn't steal HBM bandwidth from the critical path.
"""

import os

import numpy as np

import concourse.bacc as bacc
import concourse.bass as bass
import concourse.tile as tile
from concourse import mybir
from concourse.bass_utils import run_bass_kernel_spmd

F32 = mybir.dt.float32
FP16 = mybir.dt.float16
FP8 = mybir.dt.float8e4

D = 2048          # feature dim
T = 16384         # total timesteps
N_CORES = 8
TS = T // N_CORES  # timesteps per core = 2048

DT = D // 128      # 16 d-tiles of 128
KT = D // 128      # 16 k-tiles of 128
KSP = KT // 2      # 8 DoubleRow k-steps (2 k-tiles each)
TC = TS // 512     # 4 t-chunks of 512

# DoubleRowSwInterleave: weights pre-interleaved/reversed on host so the
# PE reads them contiguously (faster LDWEIGHTS than plain DoubleRow)
USE_SWI = os.environ.get("K_SWI", "0") == "1"

W2_SCALE = 32.0            # pre-scale on W2 so fp8 stays in normal range
INV_W2_SCALE = 1.0 / 32.0  # folded into the tanh activation's scale
OUT_SCALE = 2.0 ** 14      # keeps e/S in healthy fp16 range on device
INV_OUT_SCALE = 2.0 ** -14


def build_kernel(debug=False):
    nc = bacc.Bacc(None, target_bir_lowering=False, debug=debug,
                   num_devices=N_CORES)

    vt8 = nc.dram_tensor("vt8", [128, KT, TS], FP8, kind="ExternalInput")
    vt16 = nc.dram_tensor("vt16", [128, DT, TS], FP16, kind="ExternalInput")
    if USE_SWI:
        w2dr = nc.dram_tensor("w2dr", [DT, 128, KSP, 256], FP8,
                              kind="ExternalInput")
    else:
        w2dr = nc.dram_tensor("w2dr", [DT, 128, KT, 128], FP8,
                              kind="ExternalInput")
    qbv_h = nc.dram_tensor("qbv_h", [128, DT], F32, kind="ExternalInput")
    vv_h = nc.dram_tensor("vv_h", [128, DT], F32, kind="ExternalInput")
    outd = nc.dram_tensor("outd", [DT, 128, TS], FP16, kind="ExternalOutput")

    # sum-exp AllReduce split: [lo, hi) d-tile ranges.  Each collective is
    # latency-bound (~8-20us); they serialize on the CC core, so parts are
    # spaced so each finishes before the next triggers, and the early
    # parts' output DMAs drain during the matmul phase, keeping HBM quiet
    # for the tail collective.
    AR_PARTS = [(0, 10), (10, 13), (13, 16)]

    with tile.TileContext(nc) as tc:
        with (
            tc.tile_pool(name="const", bufs=1) as const_pool,
            tc.tile_pool(name="vt8p", bufs=1) as vt8_pool,
            tc.tile_pool(name="vt16p", bufs=1) as vt16_pool,
            tc.tile_pool(name="e", bufs=1) as e_pool,
            tc.tile_pool(name="w2", bufs=4) as w2_pool,
            tc.tile_pool(name="st", bufs=2) as st_pool,
            tc.tile_pool(name="osb", bufs=2) as osb_pool,
            tc.tile_pool(name="psum", bufs=2, space="PSUM") as psum_pool,
            tc.tile_pool(name="dram", bufs=1, space="DRAM") as dram_pool,
        ):
            # ---------------- constants / small vectors ----------------
            qbv = const_pool.tile([128, DT], F32)
            vv = const_pool.tile([128, DT], F32)
            rv2 = const_pool.tile([128, DT], F32)   # 2^14 / S[d]
            Sloc = const_pool.tile([128, DT], F32)  # local sum-exp

            # ---------------- warmup collective (absorbs ncfw first-use) ----
            wu_in = dram_pool.tile([1, 32], F32, name="wu_in")
            wu_out = dram_pool.tile([1, 32], F32, name="wu_out")
            wuz = const_pool.tile([1, 32], F32)
            nc.vector.memset(wuz[:, :], 0.0)
            nc.gpsimd.dma_start(wu_in[:, :], wuz[:, :])
            nc.gpsimd.collective_compute(
                "AllReduce", mybir.AluOpType.add,
                replica_groups=[list(range(N_CORES))],
                ins=[wu_in.opt()], outs=[wu_out.opt()],
            )

            nc.gpsimd.dma_start(qbv[:, :], qbv_h[:, :])
            nc.gpsimd.dma_start(vv[:, :], vv_h[:, :])

            # Startup-critical inputs (w2 dj0-3 + all of vt8, ~5 MB) spread
            # over all three DMA queues, ordered by earliest need per
            # queue; remaining W2 blocks stream behind them.
            w2_shape = [128, KSP, 256] if USE_SWI else [128, KT, 128]
            w2_tiles = {}
            for dj in range(DT):
                w2_tiles[dj] = w2_pool.tile(w2_shape, FP8, tag="w2",
                                            name=f"w2b{dj}")
            vt8_sb = vt8_pool.tile([128, KT, TS], FP8, name="vt8_sb")

            def w2_load(dj, eng):
                eng.dma_start(w2_tiles[dj][:, :, :], w2dr[dj, :, :, :])

            def vt8_load(c, eng):
                eng.dma_start(vt8_sb[:, 2 * c:2 * c + 2, :],
                              vt8[:, 2 * c:2 * c + 2, :])

            # per-queue order follows earliest need by dj0's ksp schedule
            w2_load(0, nc.sync)
            w2_load(1, nc.scalar)
            vt8_load(2, nc.gpsimd)
            vt8_load(0, nc.sync)
            vt8_load(1, nc.scalar)
            w2_load(2, nc.gpsimd)
            vt8_load(3, nc.sync)
            vt8_load(4, nc.scalar)
            w2_load(3, nc.gpsimd)
            vt8_load(6, nc.sync)
            vt8_load(7, nc.scalar)
            vt8_load(5, nc.gpsimd)
            for dj in range(4, DT):
                eng = (nc.sync, nc.scalar, nc.gpsimd)[dj % 3]
                w2_load(dj, eng)

            # vt16 [p, dj, t] on gpsimd, gated per-chunk by dependency
            # markers written inside the dj loop so it trails the matmuls
            vt16_sb = vt16_pool.tile([128, DT, TS], FP16, name="vt16_sb")

            def vt16_load(c):
                nc.gpsimd.dma_start(vt16_sb[:, 4 * c:4 * c + 4, :],
                                    vt16[:, 4 * c:4 * c + 4, :])

            e_tiles = [e_pool.tile([128, TS], FP16, name=f"e{dj}")
                       for dj in range(DT)]

            # ---------------- AllReduce plumbing ------------------------
            s_bounce = []
            for pi, (lo, hi) in enumerate(AR_PARTS):
                sin = dram_pool.tile([128, hi - lo], F32, name=f"s_in{pi}")
                sout = dram_pool.tile([128, hi - lo], F32, name=f"s_out{pi}")
                s_bounce.append((sin, sout))

            def ar_trigger(pi):
                lo, hi = AR_PARTS[pi]
                sin, sout = s_bounce[pi]
                # bounce DMA from the scalar queue: it runs right after the
                # accumulator read on the same engine, skipping a
                # cross-engine semaphore hop on the trigger path
                nc.scalar.dma_start(sin[:, :], Sloc[:, lo:hi])
                nc.gpsimd.collective_compute(
                    "AllReduce", mybir.AluOpType.add,
                    replica_groups=[list(range(N_CORES))],
                    ins=[sin.opt()], outs=[sout.opt()],
                )

            def ar_readback(pi):
                lo, hi = AR_PARTS[pi]
                sin, sout = s_bounce[pi]
                nc.gpsimd.dma_start(rv2[:, lo:hi], sout[:, :])
                nc.vector.tensor_scalar_mul(rv2[:, lo:hi], rv2[:, lo:hi],
                                            INV_OUT_SCALE)
                nc.vector.reciprocal(rv2[:, lo:hi], rv2[:, lo:hi])

            ndma = [0]

            def pass2_scale(dj, on_scalar=False):
                # e (already e*valuesT) *= 2^14/S[d], then [d, t] fp16 out
                if on_scalar:
                    osb = osb_pool.tile([128, TS], FP16, tag="osb", name="osb")
                    nc.scalar.activation(
                        osb[:, :], e_tiles[dj][:, :],
                        mybir.ActivationFunctionType.Copy,
                        bias=0.0, scale=rv2[:, dj:dj + 1])
                    src = osb
                else:
                    nc.vector.tensor_scalar(
                        out=e_tiles[dj][:, :], in0=e_tiles[dj][:, :],
                        scalar1=rv2[:, dj:dj + 1], scalar2=None,
                        op0=mybir.AluOpType.mult)
                    src = e_tiles[dj]
                ndma[0] += 1
                eng = nc.sync if ndma[0] % 2 == 0 else nc.scalar
                eng.dma_start(outd[dj, :, :], src[:, :])

            # ---------------- pass 1: matmul + tanh + exp + e*values ----
            y_cursor = [0]      # next dj whose e *= valuesT is pending
            vt16_avail = 0      # vt16 chunks emitted so far (in d-tiles)
            for dj in range(DT):
                ps = psum_pool.tile([128, TS], F32, tag="ps", name=f"ps{dj % 2}")
                for ksp in range(KSP):
                    if USE_SWI:
                        lhsT = w2_tiles[dj][:, ksp, :].rearrange(
                            "p (m two) -> p two m", two=2)
                        pm = mybir.MatmulPerfMode.DoubleRowSwInterleave
                    else:
                        lhsT = w2_tiles[dj][:, 2 * ksp:2 * ksp + 2, :]
                        pm = mybir.MatmulPerfMode.DoubleRow
                    for tci in range(TC):
                        nc.tensor.matmul(
                            ps[:, tci * 512:(tci + 1) * 512],
                            lhsT,
                            vt8_sb[:, 2 * ksp:2 * ksp + 2,
                                   tci * 512:(tci + 1) * 512],
                            start=(ksp == 0), stop=(ksp == KSP - 1),
                            perf_mode=pm,
                        )
                st = st_pool.tile([128, TS], FP16, tag="st", name="st")
                nc.scalar.activation(
                    st[:, :], ps[:, :],
                    mybir.ActivationFunctionType.Tanh,
                    bias=qbv[:, dj:dj + 1], scale=INV_W2_SCALE,
                )
                nc.scalar.activation(
                    e_tiles[dj][:, :], st[:, :],
                    mybir.ActivationFunctionType.Exp,
                    bias=0.0, scale=vv[:, dj:dj + 1],
                    accum_out=Sloc[:, dj:dj + 1],
                )
                if dj in (2, 5, 8, 11):
                    # release the next vt16 chunk: the marker write gates
                    # the DMA (WAW) behind the dj loop so it doesn't steal
                    # startup HBM bandwidth; each load is emitted before
                    # any e*values read of its range (RAW).
                    c = (dj - 2) // 3
                    nc.scalar.copy(vt16_sb[:, 4 * c:4 * c + 1, 0:1],
                                   qbv[:, 0:1])
                    vt16_load(c)
                    vt16_avail = 4 * c + 4
                # e *= valuesT (no denom needed) while TensorE grinds on
                while y_cursor[0] <= dj and y_cursor[0] < vt16_avail:
                    j = y_cursor[0]
                    nc.vector.tensor_mul(e_tiles[j][:, :], e_tiles[j][:, :],
                                         vt16_sb[:, j, :])
                    y_cursor[0] += 1

                if dj == 9:
                    ar_trigger(0)
                elif dj == 10:
                    ar_readback(0)
                elif dj == 11:
                    for j in range(0, 5):
                        pass2_scale(j)
                elif dj == 12:
                    ar_trigger(1)
                    for j in range(5, 10):
                        pass2_scale(j)
                elif dj == 14:
                    ar_readback(1)
                    for j in range(10, 13):
                        pass2_scale(j)
                elif dj == DT - 1:                # dj 15
                    ar_trigger(2)

            ar_readback(2)
            for j in range(13, 16):
                pass2_scale(j)

    nc.compile()
    return nc


_NC_CACHE = None


def _get_nc():
    global _NC_CACHE
    if _NC_CACHE is None:
        _NC_CACHE = build_kernel()
    return _NC_CACHE


def make_in_maps(query, values, v, W1_w, W1_b, W2_w, W2_b):
    import ml_dtypes
    qb = (query @ W1_w.T + W1_b + W2_b).astype(np.float32)
    qbv_np = np.ascontiguousarray(qb.reshape(DT, 128).T)
    vv_np = np.ascontiguousarray(v.reshape(DT, 128).T)
    # [dj, p, ks, m] = 32*W2[dj*128+m, ks*128+p]
    w2dr_np = np.ascontiguousarray(
        (W2_w.T * W2_SCALE).reshape(KT, 128, DT, 128).transpose(2, 1, 0, 3)
        .astype(ml_dtypes.float8_e4m3))
    if USE_SWI:
        # per (dj, p, ksp): [A127, B127, ..., A0, B0] where A/B are the m
        # columns of k-tiles 2*ksp / 2*ksp+1, m descending
        a = w2dr_np.reshape(DT, 128, KSP, 2, 128)[..., ::-1]
        w2dr_np = np.ascontiguousarray(
            a.transpose(0, 1, 2, 4, 3).reshape(DT, 128, KSP, 256))
    in_maps = []
    for c in range(N_CORES):
        valsT = values[c * TS:(c + 1) * TS].T          # [D, TS]
        base = np.ascontiguousarray(
            valsT.reshape(DT, 128, TS).transpose(1, 0, 2))  # [p, j, t]
        in_maps.append({
            "vt8": base.astype(ml_dtypes.float8_e4m3),
            "vt16": base.astype(np.float16),
            "w2dr": w2dr_np,
            "qbv_h": qbv_np,
            "vv_h": vv_np,
        })
    return in_maps


def kernel(query, values, v, W1_w, W1_b, W2_w, W2_b, _trace=False,
           _trace_kwargs=None):
    query = np.asarray(query, np.float32)
    values = np.asarray(values, np.float32)
    v = np.asarray(v, np.float32)
    W1_w = np.asarray(W1_w, np.float32)
    W1_b = np.asarray(W1_b, np.float32)
    W2_w = np.asarray(W2_w, np.float32)
    W2_b = np.asarray(W2_b, np.float32)

    nc = _get_nc()
    in_maps = make_in_maps(query, values, v, W1_w, W1_b, W2_w, W2_b)
    res = run_bass_kernel_spmd(
        nc, in_maps, core_ids=list(range(N_CORES)),
        trace=_trace, **(_trace_kwargs or {}),
    )
    shards = []
    for om in res.results:
        o = np.asarray(om["outd"])                      # [DT, 128, TS] fp16
        o = np.transpose(o, (2, 0, 1)).reshape(TS, D)   # [t, d]
        shards.append(o.astype(np.float32) * INV_OUT_SCALE)
    out = np.concatenate(shards, axis=0)
    if _trace:
        return out, res
    return out


if __name__ == "__main__":
    nc = build_kernel()
    print("compiled OK")
